# revision 15
# baseline (speedup 1.0000x reference)
"""Conv2d 3x3 via ci-packed K + 4-way concurrent col-strip matmuls (v2).

Mapping (per core, H-shard of 512 rows + halos, W padded host-side):
  - 30-row output blocks. Moving operand: [K=128, N] where partition
    32*ci + j holds input row r0+j of channel ci (j in [0,32)).
  - Stationary per (co, dx): [128, 30] band, entry (32ci+j, m) =
    k[co, ci, j-m, dx]. Output channel co is computed at PSUM partitions
    [32co, 32co+30) via tile_position=(0, 32co) - the four co matmuls of a
    round target different 32-column strips of the PE array and execute
    concurrently (one 512-col stream serves all 4 co).
  - Rounds per (block, W-half): dx(3) x wc(4), 4 concurrent MMs each,
    accumulating dx into the per-wc PSUM bank regions.
  - 512 = 17*30 + 2: block 17 overlaps (rows 482..512) and only its last
    2 rows are written out, so every matmul keeps the full-size footprint
    (no tiny tail matmuls, HAM stays warm).
"""

import numpy as np

import concourse.bass as bass
import concourse.tile as tile
from concourse import bacc, mybir
from concourse.bass_utils import run_bass_kernel_spmd

N_CORES = 8
C = 4
H = 4096
W = 4096
SH = H // N_CORES          # 512 output rows per core
YB = 30                    # output rows per block
NBLK = 18                  # 17 regular + 1 overlapping tail block
WC = 512
WHALF = 2048

MM_DT = mybir.dt.bfloat16
F32 = mybir.dt.float32

_CACHE = {}


def _r0(b: int) -> int:
    return YB * b if b < NBLK - 1 else SH - YB  # block 17 overlaps: rows 482..512


def _build_program():
    nc = bacc.Bacc(
        "TRN2", target_bir_lowering=False, debug=False, num_devices=N_CORES
    )

    xs_d = nc.dram_tensor("xs", [C, SH + 2, W + 2], MM_DT, kind="ExternalInput")
    bands_d = nc.dram_tensor("bands", [128, 12 * YB], MM_DT, kind="ExternalInput")
    ys_d = nc.dram_tensor("ys", [C, SH, W], F32, kind="ExternalOutput")

    xs = xs_d.ap()
    ys = ys_d.ap()

    with tile.TileContext(nc) as tc:
        with (
            tc.tile_pool(name="bp", bufs=1) as bpool,
            tc.tile_pool(name="xp", bufs=5) as xpool,
            tc.tile_pool(name="op", bufs=3) as opool,
            tc.tile_pool(name="pp", bufs=8, space=bass.MemorySpace.PSUM) as ppool,
        ):
            bt = bpool.tile([128, 12 * YB], MM_DT, tag="bands", name="bt")
            nc.sync.dma_start(out=bt[:], in_=bands_d.ap()[:])

            for b in range(NBLK):
                r0 = _r0(b)
                xt = xpool.tile([128, W + 2], MM_DT, tag="xt", name="xt")
                for ci in range(C):
                    eng = nc.scalar if ci < 2 else nc.sync
                    eng.dma_start(
                        out=xt[32 * ci : 32 * ci + 32, :],
                        in_=xs[ci, r0 : r0 + 32, :],
                    )
                otw = opool.tile([128, W], F32, tag="otw", name="otw")
                for wh in range(2):
                    c0 = WHALF * wh
                    pss = [
                        ppool.tile([128, WC], F32, tag="ps", name=f"ps{i}")
                        for i in range(WHALF // WC)
                    ]
                    for dx in range(3):
                        for wc in range(WHALF // WC):
                            s = c0 + WC * wc
                            for co in range(C):
                                band = bt[:, (co * 3 + dx) * YB : (co * 3 + dx + 1) * YB]
                                nc.tensor.matmul(
                                    pss[wc][32 * co : 32 * co + YB, :],
                                    band,
                                    xt[:, s + dx : s + dx + WC],
                                    start=(dx == 0),
                                    stop=(dx == 2),
                                    tile_position=(0, 32 * co),
                                    skip_group_check=True,
                                )
                    for wc in range(WHALF // WC):
                        s = c0 + WC * wc
                        nc.vector.tensor_copy(otw[:, s : s + WC], pss[wc][:])
                # output DMA: block 17 writes only its last 2 rows
                lo = 0 if b < NBLK - 1 else YB - 2
                for co in range(C):
                    eng = nc.sync if co < 2 else nc.scalar
                    eng.dma_start(
                        out=ys[co, r0 + lo : r0 + YB, :],
                        in_=otw[32 * co + lo : 32 * co + YB, :],
                    )

    nc.compile()
    return nc


def _make_bands(kw: np.ndarray):
    import ml_dtypes

    bands = np.zeros((128, 12 * YB), dtype=np.float32)
    for co in range(C):
        for dx in range(3):
            col0 = (co * 3 + dx) * YB
            for ci in range(C):
                for dy in range(3):
                    m = np.arange(YB)
                    bands[32 * ci + m + dy, col0 + m] = kw[co, ci, dy, dx]
    return bands.astype(ml_dtypes.bfloat16)


def _prep_inputs(x: np.ndarray, kw: np.ndarray) -> list[dict]:
    import ml_dtypes

    bdt = ml_dtypes.bfloat16
    xpad = np.zeros((C, H + 2, W + 2), dtype=bdt)
    xpad[:, 1 : H + 1, 1 : W + 1] = x.astype(bdt)
    bands = _make_bands(kw)
    return [
        {
            "xs": np.ascontiguousarray(xpad[:, SH * c : SH * c + SH + 2, :]),
            "bands": bands,
        }
        for c in range(N_CORES)
    ]


def kernel(x: np.ndarray, kernel: np.ndarray) -> np.ndarray:
    x = np.asarray(x, dtype=np.float32)
    kw = np.asarray(kernel, dtype=np.float32)

    if "nc" not in _CACHE:
        _CACHE["nc"] = _build_program()
    nc = _CACHE["nc"]

    in_maps = _prep_inputs(x, kw)
    res = run_bass_kernel_spmd(nc, in_maps, list(range(N_CORES)))
    out = np.concatenate([res.results[c]["ys"] for c in range(N_CORES)], axis=1)
    return out
